# revision 71
# baseline (speedup 1.0000x reference)
"""Trainium2 Bass kernel for nn_BCMEmulator (TCN emulator).

Model: 5-block dilated-causal-conv TCN (CH=64, K=3, dils 1,2,4,8,16) over
(B=128, T=1024) + pointwise heads (pet/pck softplus, aet sigmoid gate, cwd).

Strategy (pure data parallel, 8 cores x 16 sequences):
 - Each core processes 16 sequences as 8 "pairs". A pair packs 2 sequences
   into the 128 SBUF partitions: rows 0-63 = seq A channels, 64-127 = seq B.
 - Every conv tap is one matmul (K=128 contraction = 2x64 channels,
   block-diagonal weights, M=128 = 2x64 output channels, N=512 time cols).
   Causal dilation is a column offset into a left-zero-padded SBUF tensor.
 - float32r matmuls: 1 PE cycle/row for N>=256 (bf16 speed), ~1.3e-4 rel err.
 - ReLU on ScalarE (free per-partition bias), residual add fused on VectorE
   via scalar_tensor_tensor (f = max(psB,0) + f) when biases are zero.
 - softplus = ln(1+exp(.)), sigmoid(z) = exp(-ln(1+exp(-z))): only the
   natural_log_exp_and_others ACT table set is used (no table switches).
"""
import sys

sys.path.insert(0, "/opt/trn_rl_repo")

import numpy as np

import bass_rust
import concourse.bacc as bacc
import concourse.bass as bass
import concourse.tile as tile
from concourse import mybir
from concourse.bass_utils import run_bass_kernel_spmd

B, T = 128, 1024
C_IN, EMB = 15, 8
CH = 64
DILS = [1, 2, 4, 8, 16]
CT = C_IN + EMB              # 23 input channels after fveg concat
NCORES = 8
BPC = B // NCORES            # 16 sequences per core
NPAIR = BPC // 2             # 8 pairs per core
P0 = 2 * DILS[-1]            # 32 left-pad columns (max lookback)
PADT = P0 + T
TT = 512                     # matmul free-dim tile (one PSUM bank of fp32)
NTT = T // TT

F32R = mybir.dt.float32r
F32 = mybir.dt.float32
BF16 = mybir.dt.bfloat16
F8 = mybir.dt.float8e4
AF = mybir.ActivationFunctionType
ALU = mybir.AluOpType
DRMODE = mybir.MatmulPerfMode.DoubleRow

_PROGRAM_CACHE = {}


def dr_ap(a, koff):
    """Insert a 2-long k-tile dim with stride koff into a 2-dim AP: k-tile 0
    = the slice itself, k-tile 1 = the same slice offset by koff elements.
    With koff=-d the second k-tile is the data shifted right by the conv
    dilation (two taps per half-rate fp8 DR instruction, no copies); with
    koff = the x/delta-x plane stride it pairs a tensor with its fp8
    quantization residual (error-compensated matmul)."""
    a = a.copy()
    dims = a.ap.to_list()
    a.ap = bass_rust.VecI64Pair([dims[0], [koff, 2], dims[1]])
    return a


def dr_rhs(src, lo, n, koff):
    return dr_ap(src[:, lo:lo + n], koff)


def _pin_act_table():
    """Force every ACT instruction onto natural_log_exp_and_others (which
    contains Relu+Exp+Ln): the greedy per-instruction set picker otherwise
    thrashes Relu/Exp->set0, Ln->set5, inserting ~33 table loads (~2.7us
    each, serializing ScalarE). Membership is edited, order preserved, so
    emitted act_func_set_ids still index act_info.json correctly."""
    import concourse.hw_specs as hw_specs
    if getattr(bacc.get_activation_tables, "_pinned", False):
        return
    orig = bacc.get_activation_tables
    mine = {AF.Relu, AF.Exp, AF.Ln}

    def patched(arch):
        tabs = orig(arch)
        return {
            name: (set(fns) if name == "natural_log_exp_and_others"
                   else set(fns) - mine)
            for name, fns in tabs.items()
        }

    patched._pinned = True
    bacc.get_activation_tables = patched
    hw_specs_patched = patched
    del hw_specs_patched


def build_program(zero_bb):
    _pin_act_table()
    """Build + compile the per-core Bass program.

    zero_bb: tuple of 4 bools — whether bb[i] is all-zero (enables the fused
    DVE relu+residual-add; otherwise an extra ACT relu-with-bias is emitted).
    """
    nc = bacc.Bacc("TRN2", target_bir_lowering=False, debug=False,
                   num_devices=NCORES)

    xin_d = nc.dram_tensor("xin", [NPAIR, 128, 2 + T], F8, kind="ExternalInput")
    w08_d = nc.dram_tensor("w08", [128, 3, 2, 128], F8, kind="ExternalInput")
    wk_d = nc.dram_tensor("wk", [64, 12, 64], BF16, kind="ExternalInput")
    wk8_d = nc.dram_tensor("wk8", [128, 5, 2, 2, 128], F8, kind="ExternalInput")
    whpa_d = nc.dram_tensor("whpa", [128, 8, 128], BF16, kind="ExternalInput")
    wg_d = nc.dram_tensor("wg", [128, 2, 64], F32R, kind="ExternalInput")
    bias_d = nc.dram_tensor("bias", [128, 11], F32, kind="ExternalInput")
    bh_d = nc.dram_tensor("bh", [128, 1], F32, kind="ExternalInput")
    out_d = {
        nm: nc.dram_tensor(nm, [BPC, T], F32, kind="ExternalOutput")
        for nm in ("pet", "pck", "aet", "cwd")
    }

    with tile.TileContext(nc) as tc:
        with (
            tc.tile_pool(name="wpool", bufs=1) as wpool,
            tc.tile_pool(name="xpool", bufs=3) as xpool,
            tc.tile_pool(name="fpool", bufs=6) as fpool,
            tc.tile_pool(name="hpool", bufs=6) as hpool,
            tc.tile_pool(name="spool", bufs=2) as spool,
            tc.tile_pool(name="pspool", bufs=1, space=bass.MemorySpace.PSUM) as ps,
        ):
            w08_sb = wpool.tile([128, 3, 2, 128], F8)
            wk_sb = wpool.tile([128, 12, 128], BF16)
            wk8_sb = wpool.tile([128, 5, 2, 2, 128], F8)
            whpa_sb = wpool.tile([128, 8, 128], BF16)
            wg_sb = wpool.tile([128, 2, 64], F32R)
            bias_sb = wpool.tile([128, 11], F32)
            bh_sb = wpool.tile([128, 1], F32)
            # wk ships as (64,12,64); the block-diagonal (128,12,128) lhsT is
            # assembled on-device: zero the tile once, then 2 DMAs fill the
            # diagonal quadrants (weights are static, so this runs once)
            nc.vector.memset(wk_sb, 0.0)

            # warm the ACT function table at t=0: the first real ACT op
            # otherwise pays the ~1.3us LoadActFuncSet on the critical path
            warm = wpool.tile([1, 8], F32)
            nc.vector.memset(warm, 0.0)
            warm2 = wpool.tile([1, 8], F32)
            nc.scalar.activation(out=warm2, in_=warm, func=AF.Relu, scale=1.0)

            # preload all pair inputs; w08 + pair 0's xin go first on their
            # queues so conv0a can start as early as possible
            nc.gpsimd.dma_start(out=w08_sb, in_=w08_d[:])
            xins = []
            for p in range(NPAIR):
                xin_sb = xpool.tile([128, 2 + T], F8, tag=f"xin{p}",
                                    name=f"xin_sb{p}", bufs=1)
                eng = nc.sync if p % 2 == 0 else nc.gpsimd
                eng.dma_start(out=xin_sb, in_=xin_d[p])
                xins.append(xin_sb)
                if p == 0:
                    nc.sync.dma_start(out=bias_sb, in_=bias_d[:])
                if p == 1:
                    nc.gpsimd.dma_start(out=wk8_sb, in_=wk8_d[:])
                if p == 2:
                    nc.sync.dma_start(out=bh_sb, in_=bh_d[:])
                    nc.sync.dma_start(out=wk_sb[0:64, :, 0:64], in_=wk_d[:])
                    nc.sync.dma_start(out=wk_sb[64:128, :, 64:128], in_=wk_d[:])
                    nc.sync.dma_start(out=whpa_sb, in_=whpa_d[:])
                    nc.sync.dma_start(out=wg_sb, in_=wg_d[:])

            GRP = 3

            def conv_taps(psum, lhsT_of_j, src, d, base_k=128):
                """3-tap dilated causal conv: psum += sum_j W_j @ src shifted
                right by s=(2-j)*d. Causal zero-padding falls out of PSUM
                has_written semantics: the shift-0 tap goes first (start=True,
                full width, clears the bank); shifted taps then accumulate
                into fully-written regions, leaving the left edge untouched
                where their input would be out of range. Column tile 0 is
                finished before tile 1 starts so downstream reads of the
                first half unblock early."""
                for t in range(NTT):
                    lo = t * TT
                    for j in (2, 1, 0):
                        s = (2 - j) * d
                        out_lo = lo + (s if t == 0 else 0)
                        nc.tensor.matmul(
                            psum[:, out_lo:lo + TT],
                            lhsT_of_j(j),
                            src[:base_k, out_lo - s:lo + TT - s],
                            start=(j == 2),
                            stop=(j == 0),
                        )

            def conv_f8(psum, c, src, d):
                """fp8 DoubleRow 3-tap conv over a left-zero-padded source
                tile (data at cols [P0:P0+T]): wk8_sb[:, c, 0] holds k-tiles
                [tap2, tap1], wk8_sb[:, c, 1] holds [tap0, zero]. Each DR
                instruction streams two taps at half rate; causal padding
                falls out of the zeroed pad columns, so every instruction is
                full width (start=True covers the whole bank as HW needs)."""
                la = wk8_sb[:, c, 0, :, :]
                lb = wk8_sb[:, c, 1, :, :]
                for t in range(NTT):
                    lo = P0 + t * TT
                    nc.tensor.matmul(psum[:, t * TT:(t + 1) * TT], la,
                                     dr_rhs(src, lo, TT, -d),
                                     start=True, stop=False, perf_mode=DRMODE)
                    nc.tensor.matmul(psum[:, t * TT:(t + 1) * TT], lb,
                                     dr_rhs(src, lo - 2 * d, TT, d),
                                     start=False, stop=True, perf_mode=DRMODE)

            def h1_tile(name):
                """fp8 h1 tile with P0 zeroed pad columns (zeroed on the
                mostly-idle Pool engine, ~0.1us per tile)."""
                h1 = hpool.tile([128, P0 + T], F8, tag="h1", name=name)
                nc.gpsimd.memset(h1[:, 0:P0], 0.0)
                return h1

            def stt_split(out, in0, scalar, in1, op0, op1):
                """scalar_tensor_tensor split by column half so the first
                half unblocks downstream reads early (GPSIMD cannot touch
                PSUM on hardware, so both halves run on DVE)."""
                for t in range(NTT):
                    sl = slice(t * TT, (t + 1) * TT)
                    nc.vector.scalar_tensor_tensor(
                        out=out[:, sl], in0=in0[:, sl], scalar=scalar,
                        in1=in1[:, sl], op0=op0, op1=op1)

            # Two-level software pipeline, stage-interleaved emission:
            #  - pairs are processed in groups of GRP=2; within each stage the
            #    per-pair ops are emitted round-robin so every engine has
            #    same-stage work from both pairs queued;
            #  - the previous group's head stages are drained between the
            #    current group's block phases, so head transcendentals overlap
            #    the next group's conv matmuls and only the last group's heads
            #    form the kernel tail.
            # PSUM: tag p%4 (one (128,1024) = 2-bank slot per pair), so
            # consecutive groups use disjoint tag pairs {0,1}/{2,3}.
            st = {}

            def blk0_convA(grp):
                # fp8 DR conv over xin: partitions stack [x8(46); dx8(46);
                # zeros] so the input-quantization residual is compensated
                # for free ("w.x + w.dx" via duplicated weight rows); K=128
                # as DoubleRow requires. Data at cols [2:2+T], 2 zero pad
                # cols handle causality. Per tile: DR[tap2,tap1@-1] +
                # DR[tap0@-2, delta-tap1@-1].
                for p in grp:
                    psA = ps.tile([128, T], F32, tag=f"ps{p % 3}",
                                  name=f"psA0_{p}")
                    for t in range(NTT):
                        lo = 2 + t * TT
                        nc.tensor.matmul(psA[:, t * TT:(t + 1) * TT],
                                         w08_sb[:, 0, :, :],
                                         dr_rhs(xins[p], lo, TT, -1),
                                         start=True, stop=False,
                                         perf_mode=DRMODE)
                        nc.tensor.matmul(psA[:, t * TT:(t + 1) * TT],
                                         w08_sb[:, 1, :, :],
                                         dr_rhs(xins[p], lo - 2, TT, 1),
                                         start=False, stop=True,
                                         perf_mode=DRMODE)
                    st[p] = {"ps": psA}
                for p in grp:
                    h1 = h1_tile(f"h1b0_{p}")
                    nc.scalar.activation(out=h1[:, P0:P0 + T], in_=st[p]["ps"],
                                         func=AF.Relu,
                                         bias=bias_sb[:, 0:1], scale=1.0)
                    st[p]["h1"] = h1

            def blk0_convB(grp):
                for p in grp:
                    psB = ps.tile([128, T], F32, tag=f"ps{p % 3}",
                                  name=f"psB0_{p}")
                    conv_f8(psB, 0, st[p]["h1"], 1)
                    st[p]["ps"] = psB
                for p in grp:
                    h2 = hpool.tile([128, T], BF16, tag="h2", name=f"h2_{p}",
                                    bufs=5)
                    nc.scalar.activation(out=h2, in_=st[p]["ps"],
                                         func=AF.Relu,
                                         bias=bias_sb[:, 1:2], scale=1.0)
                    st[p]["h2"] = h2

            def blk0_resid(grp):
                for p in grp:
                    psR = ps.tile([128, T], F32, tag=f"ps{p % 3}",
                                  name=f"psR_{p}")
                    # 1x1 resid conv: k-tiles (cols, cols) at stride 0 with
                    # lhsT [w0r, delta-w0r] -> weight-compensated (the input
                    # residual rides along the stacked partitions)
                    for t in range(NTT):
                        lo = 2 + t * TT
                        nc.tensor.matmul(
                            psR[:, t * TT:(t + 1) * TT], w08_sb[:, 2, :, :],
                            dr_rhs(xins[p], lo, TT, 0),
                            start=True, stop=True, perf_mode=DRMODE)
                    st[p]["ps"] = psR
                for p in grp:
                    f = fpool.tile([128, T], BF16, tag="f", name=f"f_{p}")
                    stt_split(f, st[p]["ps"], bias_sb[:, 2:3], st[p]["h2"],
                              ALU.add, ALU.add)
                    st[p]["f"] = f

            def blk(grp, i, d):
                for p in grp:
                    psA = ps.tile([128, T], F32, tag=f"ps{p % 3}",
                                  name=f"psA{i + 1}_{p}")
                    conv_taps(psA, lambda j: wk_sb[:, 3 * i + j, :],
                              st[p]["f"], d)
                    st[p]["ps"] = psA
                for p in grp:
                    h1 = h1_tile(f"h1_{i + 1}_{p}")
                    nc.scalar.activation(out=h1[:, P0:P0 + T],
                                         in_=st[p]["ps"], func=AF.Relu,
                                         bias=bias_sb[:, 3 + i:4 + i],
                                         scale=1.0)
                    st[p]["h1"] = h1
                for p in grp:
                    psB = ps.tile([128, T], F32, tag=f"ps{p % 3}",
                                  name=f"psB{i + 1}_{p}")
                    conv_f8(psB, 1 + i, st[p]["h1"], d)
                    st[p]["ps"] = psB
                for p in grp:
                    if zero_bb[i]:
                        stt_split(st[p]["f"], st[p]["ps"], 0.0, st[p]["f"],
                                  ALU.max, ALU.add)
                    else:
                        h2 = hpool.tile([128, T], BF16, tag="h2",
                                        name=f"h2_{i + 1}_{p}", bufs=5)
                        nc.scalar.activation(out=h2, in_=st[p]["ps"],
                                             func=AF.Relu,
                                             bias=bias_sb[:, 7 + i:8 + i],
                                             scale=1.0)
                        nc.vector.tensor_tensor(
                            out=st[p]["f"], in0=st[p]["f"],
                            in1=h2, op=ALU.add)

            # Packed heads, split in two pair-halves so the first half's
            # softplus/sigmoid chain runs mid-kernel (hidden under convs)
            # and only the second half's chain forms the kernel tail.
            # Each half packs its 8 seqs' head linears into one bank-packed
            # (64, 512) psum: column tile t lives at partition offset 32*t,
            # rows (within a 32-block) 0-7 aet-linear, 8-15 pet, 16-23 pck.
            # Pair matmuls M-pack via mostly-zero lhsT columns, accumulating
            # across the half's 4 pairs (zeros add harmlessly).
            # single (104, T) = 2-bank psum holding BOTH halves' head rows
            # at 32-aligned partition bases: pet-A 0:8, pck-A 8:16, pet-B
            # 32:40, pck-B 40:48, aet-A 64:72, aet-B 96:104. Every pair's
            # matmul M-packs the full 104 rows (zeros elsewhere accumulate
            # harmlessly), so partition count costs nothing extra.
            psH = ps.tile([128, T], F32, tag="psH", name="psH")

            def head_mm(grp):
                # every pair writes all 128 rows (zeros outside its class
                # columns accumulate harmlessly) so every matmul is a plain
                # (128,128)@(0,0) tile -- off-zero column positions trip ISA
                # checks for some dtypes/shapes
                for p in grp:
                    for t in range(NTT):
                        sl = slice(t * TT, (t + 1) * TT)
                        nc.tensor.matmul(psH[:, sl],
                                         whpa_sb[:, p, :], st[p]["f"][:, sl],
                                         start=(p == 0), stop=(p == 7),
                                         skip_group_check=True)

            def head_half_stages(h):
                """Per-(col-tile, stage) thunks for one half's head chain,
                so the caller can interleave them between conv phases."""
                rows = slice(8 * h, 8 * h + 8)
                pp = 64 + 32 * h     # pet/pck partition base for this half
                ap = 32 * h          # aet partition base for this half
                hh = {}
                out = []
                for t in range(NTT):
                    sl = slice(t * TT, (t + 1) * TT)

                    def s_sp(t=t, sl=sl):
                        e16 = spool.tile([16, TT], F32, tag="e16",
                                         name=f"e{h}{t}")
                        nc.scalar.activation(out=e16, in_=psH[pp:pp + 16, sl],
                                             func=AF.Exp,
                                             bias=bh_sb[pp:pp + 16, 0:1],
                                             scale=1.0)
                        # sp lives in a 128-row tile (rows 16+ zeroed) so the
                        # gate matmul gets the 128-row tile shape the ISA
                        # accepts at column position 64
                        sp16 = spool.tile([128, TT], F32R, tag="sp16",
                                          name=f"sp{h}{t}")
                        nc.gpsimd.memset(sp16.bitcast(F32), 0.0)
                        nc.scalar.activation(out=sp16[0:16, :], in_=e16,
                                             func=AF.Ln, bias=1.0, scale=1.0)
                        hh[t] = sp16

                    def s_pdma(t=t, sl=sl):
                        sp16 = hh[t]
                        nc.sync.dma_start(out=out_d["pet"][rows, sl],
                                          in_=sp16.bitcast(F32)[0:8, :])
                        nc.sync.dma_start(out=out_d["pck"][rows, sl],
                                          in_=sp16.bitcast(F32)[8:16, :])
                        # gate linear: aet += wpet*sp_pet + wpck*sp_pck
                        # (accumulates onto the stopped aet-lin region; psum
                        # has_written bits persist so start=False just adds)
                        # M padded to 64 (zero cols accumulate harmlessly
                        # into rows the chain already consumed): the ISA
                        # rejects 32-wide column tiles at position 64
                        nc.tensor.matmul(psH[0:64, sl], wg_sb[:, h, :],
                                         sp16, start=False, stop=True,
                                         skip_group_check=True)

                    def s_sig(t=t, sl=sl):
                        ge8 = spool.tile([8, TT], F32, tag="ge8",
                                         name=f"ge{h}{t}")
                        nc.scalar.activation(out=ge8, in_=psH[ap:ap + 8, sl],
                                             func=AF.Exp,
                                             bias=bh_sb[ap:ap + 8, 0:1],
                                             scale=-1.0)
                        gl8 = spool.tile([8, TT], F32, tag="gl8",
                                         name=f"gl{h}{t}")
                        nc.scalar.activation(out=gl8, in_=ge8, func=AF.Ln,
                                             bias=1.0, scale=1.0)
                        gg8 = spool.tile([8, TT], F32, tag="gg8",
                                         name=f"gg{h}{t}")
                        nc.scalar.activation(out=gg8, in_=gl8, func=AF.Exp,
                                             scale=-1.0)
                        hh[(t, "gg")] = gg8

                    def s_out(t=t, sl=sl):
                        sp16, gg8 = hh[t], hh[(t, "gg")]
                        aet8 = spool.tile([8, TT], F32, tag="aet8",
                                          name=f"aet{h}{t}")
                        nc.vector.tensor_tensor(out=aet8, in0=gg8,
                                                in1=sp16.bitcast(F32)[0:8, :],
                                                op=ALU.mult)
                        nc.sync.dma_start(out=out_d["aet"][rows, sl],
                                          in_=aet8)
                        cwd8 = spool.tile([8, TT], F32, tag="cwd8",
                                          name=f"cwd{h}{t}")
                        nc.vector.tensor_tensor(out=cwd8,
                                                in0=sp16.bitcast(F32)[0:8, :],
                                                in1=aet8, op=ALU.subtract)
                        nc.sync.dma_start(out=out_d["cwd"][rows, sl],
                                          in_=cwd8)

                    out.append([s_sp, s_pdma, s_sig, s_out])
                # interleave the two column tiles' chains stage-wise so ACT
                # streams continuously while the other tile's DVE/DMA overlap
                return [s for pair in zip(*out) for s in pair]

            pending = []

            def drain(n):
                for _ in range(n):
                    if pending:
                        pending.pop(0)()

            groups = [list(range(g0, min(g0 + GRP, NPAIR)))
                      for g0 in range(0, NPAIR, GRP)]
            # the next group's conv0a depends only on the preloaded xin, so
            # it is hoisted before this group's head matmuls: PE chews on it
            # while the current group's last residual STTs drain on DVE
            blk0_convA(groups[0])
            for gi, grp in enumerate(groups):
                for ph in ([lambda: blk0_convB(grp),
                            lambda: blk0_resid(grp)]
                           + [lambda i=i, d=d: blk(grp, i, d)
                              for i, d in enumerate(DILS[1:])]):
                    ph()
                    drain(1)
                if gi + 1 < len(groups):
                    blk0_convA(groups[gi + 1])
                    drain(1)
                head_mm(grp)
                drain(1)
                if min(grp) <= 3 <= max(grp):
                    pending += head_half_stages(0)
            pending += head_half_stages(1)
            drain(len(pending))

    nc.compile()
    return nc


def get_program(zero_bb):
    key = tuple(zero_bb)
    if key not in _PROGRAM_CACHE:
        _PROGRAM_CACHE[key] = build_program(key)
    return _PROGRAM_CACHE[key]


def prep_inputs(inputs):
    """Host-side packing: returns (zero_bb, shared weight map, per-core xin)."""
    g = {k: np.asarray(v) for k, v in inputs.items()}
    x = g["x"].astype(np.float32, copy=False)
    ids = g["fveg_ids"].astype(np.int64)
    emb = g["fveg_emb"].astype(np.float32, copy=False)

    fv = emb[ids]                                     # (B, EMB)
    xin = np.concatenate(
        [x, np.broadcast_to(fv[:, :, None], (B, EMB, T))], axis=1)  # (B,23,T)
    import ml_dtypes
    f8 = ml_dtypes.float8_e4m3
    x8 = xin.astype(f8)
    dx8 = (xin - x8.astype(np.float32)).astype(f8)
    xin_pad = np.zeros((B // 2, 128, 2 + T), f8)      # one row block per pair
    xp = x8.reshape(B // 2, 2 * CT, T)
    dp = dx8.reshape(B // 2, 2 * CT, T)
    xin_pad[:, 0:46, 2:] = xp
    xin_pad[:, 46:92, 2:] = dp
    xin_cores = np.ascontiguousarray(
        xin_pad.reshape(NCORES, NPAIR, 128, 2 + T))

    # fp8 DR lhsT for conv0a + 1x1 resid: group 0 = [tap2, tap1],
    # group 1 = [tap0, delta-tap1], group 2 = [w0r, delta-w0r]
    w0a, w0r = g["w0a"].astype(np.float32), g["w0r"].astype(np.float32)
    w08 = np.zeros((128, 3, 2, 128), f8)

    def q8d(w):                          # (23,64) fp8 + fp8 residual
        wq = w.astype(f8)
        return wq, (w - wq.astype(np.float32)).astype(f8)

    t2, _ = q8d(w0a[:, :, 2].T)
    t1, d1 = q8d(w0a[:, :, 1].T)
    t0, _ = q8d(w0a[:, :, 0].T)
    r0q, r0d = q8d(w0r[:, :, 0].T)
    for s in range(2):                  # seq-in-pair
        r, c = s * CT, s * 64
        for base in (0, 46):            # x8 rows, then delta-x8 rows
            w08[base + r:base + r + CT, 0, 0, c:c + 64] = t2
            w08[base + r:base + r + CT, 0, 1, c:c + 64] = t1
            w08[base + r:base + r + CT, 1, 0, c:c + 64] = t0
            w08[base + r:base + r + CT, 1, 1, c:c + 64] = d1
            w08[base + r:base + r + CT, 2, 0, c:c + 64] = r0q
            w08[base + r:base + r + CT, 2, 1, c:c + 64] = r0d

    # f32r lhsT for the convA taps of blocks 1-4 (rhs is the bf16 f stream)
    wa, wb = g["wa"].astype(np.float32), g["wb"].astype(np.float32)
    wk = np.zeros((12, 64, 64), np.float32)
    for i in range(4):
        for j in range(3):
            wk[3 * i + j] = wa[i, :, :, j].T
    wk = np.ascontiguousarray(wk.transpose(1, 0, 2)).astype(
        ml_dtypes.bfloat16)                            # (64, 12, 64)

    # fp8 block-diagonal DoubleRow lhsT for conv0b + convB of blocks 1-4:
    # [c, 0] k-tiles = [tap2, tap1]; [c, 1] = [tap0, delta-tap1]. The second
    # instruction's k-tile 1 reads the tap1-aligned columns (shift -d), so
    # the otherwise-zero slot carries tap1's fp8 quantization residual for
    # free extra precision.
    f8 = ml_dtypes.float8_e4m3
    wk8 = np.zeros((128, 5, 2, 2, 128), f8)
    convs = [g["w0b"].astype(np.float32)] + [wb[i] for i in range(4)]
    for c, w in enumerate(convs):
        w1 = w[:, :, 1].T
        d1 = (w1 - w1.astype(f8).astype(np.float32)).astype(f8)
        for s in range(2):
            r0 = 64 * s
            wk8[r0:r0 + 64, c, 0, 0, r0:r0 + 64] = w[:, :, 2].T.astype(f8)
            wk8[r0:r0 + 64, c, 0, 1, r0:r0 + 64] = w1.astype(f8)
            wk8[r0:r0 + 64, c, 1, 0, r0:r0 + 64] = w[:, :, 0].T.astype(f8)
            wk8[r0:r0 + 64, c, 1, 1, r0:r0 + 64] = d1

    pet_w = g["pet_w"].astype(np.float32)[0, :, 0]    # (64,)
    pck_w = g["pck_w"].astype(np.float32)[0, :, 0]
    aet_w = g["aet_w"].astype(np.float32)[0, :, 0]    # (66,)
    wpet, wpck = aet_w[64], aet_w[65]
    # packed-head lhsT per pair: (128, 8, 32); within a pair-half's 32-row
    # block: rows 0-7 aet-linear, 8-15 pet, 16-23 pck (row = local seq)
    whpa = np.zeros((128, 8, 128), np.float32)
    for p in range(8):
        h, q = p // 4, p % 4
        pp, ap = 64 + 32 * h, 32 * h
        for s in range(2):
            r0 = 64 * s
            whpa[r0:r0 + 64, p, ap + 2 * q + s] = aet_w[0:64]
            whpa[r0:r0 + 64, p, pp + 2 * q + s] = pet_w
            whpa[r0:r0 + 64, p, pp + 8 + 2 * q + s] = pck_w
    whpa = whpa.astype(ml_dtypes.bfloat16)
    # gate lhsT: sp16 rows (0-7 pet, 8-15 pck) -> aet rows 0-7
    wg = np.zeros((128, 2, 64), np.float32)
    for h in range(2):
        for s in range(8):
            wg[s, h, 32 * h + s] = wpet
            wg[8 + s, h, 32 * h + s] = wpck

    bcols = [g["b0a"], g["b0b"], g["b0r"]] + [g["ba"][i] for i in range(4)] \
        + [g["bb"][i] for i in range(4)]
    bias = np.stack([np.tile(c.astype(np.float32), 2) for c in bcols], axis=1)

    pet_b = float(g["pet_b"][0])
    pck_b = float(g["pck_b"][0])
    aet_b = float(g["aet_b"][0])
    bh = np.zeros((128, 1), np.float32)
    for h in range(2):
        pp, ap = 64 + 32 * h, 32 * h
        bh[pp:pp + 8, 0] = pet_b
        bh[pp + 8:pp + 16, 0] = pck_b
        bh[ap:ap + 8, 0] = -aet_b

    zero_bb = tuple(bool(np.all(g["bb"][i] == 0)) for i in range(4))
    shared = {"w08": w08, "wk": wk, "wk8": wk8, "whpa": whpa, "wg": wg,
              "bias": bias, "bh": bh}
    return zero_bb, shared, xin_cores


def run(inputs, trace=False, trace_kwargs=None):
    zero_bb, shared, xin_cores = prep_inputs(inputs)
    nc = get_program(zero_bb)
    in_maps = [
        {"xin": np.ascontiguousarray(xin_cores[c]), **shared}
        for c in range(NCORES)
    ]
    res = run_bass_kernel_spmd(nc, in_maps, core_ids=list(range(NCORES)),
                               trace=trace, **(trace_kwargs or {}))
    outs = []
    for nm in ("pet", "pck", "aet", "cwd"):
        full = np.concatenate([res.results[c][nm] for c in range(NCORES)], 0)
        outs.append(full.reshape(B, 1, T).astype(np.float32))
    return tuple(outs), res


def kernel(**inputs):
    outs, _ = run(inputs)
    return outs


def build_calib():
    """Same I/O signature as the real program, minimal compute — used by the
    bench to measure the axon relay's per-exec input-staging overhead."""
    _pin_act_table()
    nc = bacc.Bacc("TRN2", target_bir_lowering=False, debug=False,
                   num_devices=NCORES)
    xin_d = nc.dram_tensor("xin", [NPAIR, 128, 2 + T], F8,
                           kind="ExternalInput")
    w08_d = nc.dram_tensor("w08", [128, 3, 2, 128], F8,
                           kind="ExternalInput")
    wk_d = nc.dram_tensor("wk", [64, 12, 64], BF16, kind="ExternalInput")
    wk8_d = nc.dram_tensor("wk8", [128, 5, 2, 2, 128], F8,
                           kind="ExternalInput")
    whpa_d = nc.dram_tensor("whpa", [128, 8, 128], BF16, kind="ExternalInput")
    wg_d = nc.dram_tensor("wg", [128, 2, 64], F32R, kind="ExternalInput")
    bias_d = nc.dram_tensor("bias", [128, 11], F32, kind="ExternalInput")
    bh_d = nc.dram_tensor("bh", [128, 1], F32, kind="ExternalInput")
    out_d = {
        nm: nc.dram_tensor(nm, [BPC, T], F32, kind="ExternalOutput")
        for nm in ("pet", "pck", "aet", "cwd")
    }
    with tile.TileContext(nc) as tc:
        with tc.tile_pool(name="sb", bufs=2) as sb:
            t = sb.tile([BPC, T], F8)
            nc.sync.dma_start(out=t, in_=xin_d[0, 0:BPC, 2:2 + T])
            t2 = sb.tile([BPC, T], F32)
            nc.vector.tensor_scalar_mul(out=t2, in0=t, scalar1=1.0)
            for nm in ("pet", "pck", "aet", "cwd"):
                nc.sync.dma_start(out=out_d[nm][:], in_=t2)
    nc.compile()
    return nc

